# revision 32
# baseline (speedup 1.0000x reference)
"""Chebyshev graph-conv kernel for Trainium2 (8 NeuronCores, SPMD).

Math: out[b,o,m,t] = sum_{k,c,n} T[k,n,m] * x[b,c,n,t] * Theta[k,c,o]
with T the Chebyshev polynomials of the normalized adjacency (n=24, K=3).

The whole operator collapses into a single 768x768 matrix
    W[(c,n),(o,m)] = sum_k Theta[k,c,o] * T[k,n,m]
applied per batch element to x[b] viewed as (c*n, t) = (768, 512):
    out[b](o*24+m, t) = W.T-contract over rows -> exactly one matmul chain.

W is tiny and computed on host from adj/Theta; x is read once and out
written once. Data-parallel over batch: 64 -> 8 per core. x and W ship as
fp16 (the PE multiplies 16-bit operands at full rate with fully-hidden
weight loads, and fp16 inputs halve HBM read traffic); PSUM accumulation
and the output path are full fp32. Per core: 8 batch elements, each a 6x6
chain of [128,128]x[128,512] matmuls accumulated in PSUM.
"""

import numpy as np

import concourse.mybir as mybir
from concourse import bacc, tile
from concourse.bass import _add_dep_helper
from concourse.bass_utils import run_bass_kernel_spmd

N_CORES = 8
B, C, NV, T = 64, 32, 24, 512
K = 3
O = 32
CN = C * NV   # 768 contraction rows
OM = O * NV   # 768 output rows
BP = B // N_CORES  # 8 batch elements per core
P = 128
NBLK = CN // P  # 6

_compiled_nc = None
last_result = None  # BassKernelResults from the most recent run (for test.py)


def _build_nc():
    f32 = mybir.dt.float32
    f32r = mybir.dt.float32r
    f16 = mybir.dt.float16
    nc = bacc.Bacc("TRN2", target_bir_lowering=False, debug=False,
                   num_devices=N_CORES)
    # All-fp16 matmul inputs (walrus forbids mixing 32/16-bit operands): a
    # 2-byte weight load (with FWL) fully hides under the previous matmul
    # where the 4-byte fp32r fused load could not (216ns/MM vs 247). x and W
    # ship fp16 from host — the device matmul consumes fp16 either way, so
    # host-casting is bit-equivalent and halves the HBM read traffic. PSUM
    # accumulation and the output path stay full fp32. fp16 (m10) rounding
    # on the operands costs ~4e-4 absmax rel err (resid_var ~1e-7).
    xs = nc.dram_tensor("xs", [BP, CN, T], f16, kind="ExternalInput")
    w = nc.dram_tensor("w", [CN, OM], f16, kind="ExternalInput")
    out = nc.dram_tensor("out", [BP, OM, T], f32, kind="ExternalOutput")

    wr = w[:].rearrange("(i p) m -> p i m", p=P)
    del f32r  # unused in the all-fp16 variant

    with tile.TileContext(nc) as tc:
        with (
            tc.tile_pool(name="wpool", bufs=1) as wpool,
            tc.tile_pool(name="xpool", bufs=5) as xpool,
            tc.tile_pool(name="opool", bufs=6) as opool,
            tc.tile_pool(name="psum", bufs=8, space="PSUM") as psum_pool,
        ):
            # W as 6 chunks of [128 (cn), 768 (om)], all in one SBUF tile.
            # Tiles are float32r (bit-identical to f32; matmul runs at full
            # rate with FP22 multiply) — BIR verifier requires the producer
            # of an fp32r matmul operand to be typed fp32r.
            # Loads go on the Sync HWDGE ring; stores on the Scalar HWDGE
            # ring so stores never head-of-line-block loads. W and the first
            # batch's x are loaded chunk-wise so the first matmul only waits
            # for chunk 0 of each (~0.6 MB) instead of the full 3.75 MB.
            wt = wpool.tile([P, NBLK, OM], f16)
            xt0 = xpool.tile([P, NBLK, T], f16)
            xr0 = xs[0].rearrange("(i p) t -> p i t", p=P)
            for i in range(0, NBLK, 2):
                nc.sync.dma_start(wt[:, i:i + 2, :], wr[:, i:i + 2, :])
                nc.sync.dma_start(xt0[:, i:i + 2, :], xr0[:, i:i + 2, :])

            xts = [xt0]
            loads = [None]
            for b in range(1, BP):
                xt = xpool.tile([P, NBLK, T], f16, tag="xt0")
                xr = xs[b].rearrange("(i p) t -> p i t", p=P)
                loads.append(nc.sync.dma_start(xt[:], xr))
                xts.append(xt)

            for b in range(BP):
                xt = xts[b]
                ot = opool.tile([P, NBLK, T], f32)
                orr = out[b].rearrange("(j p) t -> p j t", p=P)
                # Hold batch b's stores until the load of batch b+2 completes:
                # loads sprint at full HBM rate early instead of round-robin
                # sharing with stores, so the last x arrives well before the
                # PE needs it; the store backlog drains in the tail where HBM
                # would otherwise idle. opool=6 keeps copies/PSUM unblocked.
                dep = loads[b + 2] if b + 2 < BP else None
                for j in range(NBLK):
                    ps = psum_pool.tile([P, T], f32)
                    for i in range(NBLK):
                        nc.tensor.matmul(
                            ps[:],
                            wt[:, i, j * P:(j + 1) * P],
                            xt[:, i, :],
                            start=(i == 0),
                            stop=(i == NBLK - 1),
                        )
                    nc.vector.tensor_copy(ot[:, j, :], ps[:])
                    st = nc.scalar.dma_start(orr[:, j, :], ot[:, j, :])
                    if dep is not None:
                        _add_dep_helper(
                            st.ins, dep.ins, sync=True,
                            reason="hold stores behind prefetch loads",
                        )

    nc.compile()
    return nc


def _combined_operator(adj: np.ndarray, Theta: np.ndarray) -> np.ndarray:
    """W[(c,n),(o,m)] = sum_k Theta[k,c,o] * T[k,n,m], fp32, shape (768,768)."""
    adj = np.asarray(adj).astype(np.float32)
    Theta = np.asarray(Theta)
    d = adj.sum(axis=1)
    d_inv_sqrt = np.where(d > 0, 1.0 / np.sqrt(d), 0.0).astype(np.float32)
    L = (adj * d_inv_sqrt[None, :]).T * d_inv_sqrt[None, :]
    Ts = [np.eye(NV, dtype=np.float32), L.astype(np.float32)]
    for _ in range(2, K):
        Ts.append((2.0 * L @ Ts[-1] - Ts[-2]).astype(np.float32))
    Tcheb = np.stack(Ts[:K])  # (K, n, m)
    W = np.einsum("kco,knm->cnom", Theta.astype(np.float32), Tcheb)
    return np.ascontiguousarray(W.reshape(CN, OM), dtype=np.float16)


def kernel(x: np.ndarray, adj: np.ndarray, Theta: np.ndarray) -> np.ndarray:
    global _compiled_nc, last_result
    if _compiled_nc is None:
        _compiled_nc = _build_nc()
    nc = _compiled_nc

    W = _combined_operator(adj, Theta)
    # x: (64, 32, 24, 512) -> per-core shard [8, 768, 512], fp16 (the device
    # matmul consumes fp16 regardless; casting host-side halves HBM reads)
    xf = np.asarray(x).astype(np.float16).reshape(B, CN, T)
    in_maps = [
        {"xs": np.ascontiguousarray(xf[c * BP:(c + 1) * BP]), "w": W}
        for c in range(N_CORES)
    ]
    res = run_bass_kernel_spmd(nc, in_maps, core_ids=list(range(N_CORES)))
    last_result = res
    out = np.concatenate([r["out"] for r in res.results], axis=0)
    return np.ascontiguousarray(out.reshape(B, O, NV, T))


# revision 33
# speedup vs baseline: 1.0333x; 1.0333x over previous
"""Chebyshev graph-conv kernel for Trainium2 (8 NeuronCores, SPMD).

Math: out[b,o,m,t] = sum_{k,c,n} T[k,n,m] * x[b,c,n,t] * Theta[k,c,o]
with T the Chebyshev polynomials of the normalized adjacency (n=24, K=3).

The whole operator collapses into a single 768x768 matrix
    W[(c,n),(o,m)] = sum_k Theta[k,c,o] * T[k,n,m]
applied per batch element to x[b] viewed as (c*n, t) = (768, 512):
    out[b](o*24+m, t) = W.T-contract over rows -> exactly one matmul chain.

W is tiny and computed on host from adj/Theta; x is read once and out
written once. Data-parallel over batch: 64 -> 8 per core. x and W ship as
fp16 (the PE multiplies 16-bit operands at full rate with fully-hidden
weight loads, and fp16 inputs halve HBM read traffic); PSUM accumulation
and the output path are full fp32. Per core: 8 batch elements, each a 6x6
chain of [128,128]x[128,512] matmuls accumulated in PSUM.
"""

import numpy as np

import concourse.mybir as mybir
from concourse import bacc, tile
from concourse.bass import _add_dep_helper
from concourse.bass_utils import run_bass_kernel_spmd

N_CORES = 8
B, C, NV, T = 64, 32, 24, 512
K = 3
O = 32
CN = C * NV   # 768 contraction rows
OM = O * NV   # 768 output rows
BP = B // N_CORES  # 8 batch elements per core
P = 128
NBLK = CN // P  # 6

_compiled_nc = None
last_result = None  # BassKernelResults from the most recent run (for test.py)


def _build_nc():
    f32 = mybir.dt.float32
    f32r = mybir.dt.float32r
    f16 = mybir.dt.float16
    nc = bacc.Bacc("TRN2", target_bir_lowering=False, debug=False,
                   num_devices=N_CORES)
    # All-fp16 matmul inputs (walrus forbids mixing 32/16-bit operands): a
    # 2-byte weight load (with FWL) fully hides under the previous matmul
    # where the 4-byte fp32r fused load could not (216ns/MM vs 247). x and W
    # ship fp16 from host — the device matmul consumes fp16 either way, so
    # host-casting is bit-equivalent and halves the HBM read traffic. PSUM
    # accumulation and the output path stay full fp32. fp16 (m10) rounding
    # on the operands costs ~4e-4 absmax rel err (resid_var ~1e-7).
    xs = nc.dram_tensor("xs", [BP, CN, T], f16, kind="ExternalInput")
    w = nc.dram_tensor("w", [CN, OM], f16, kind="ExternalInput")
    out = nc.dram_tensor("out", [BP, OM, T], f32, kind="ExternalOutput")

    wr = w[:].rearrange("(i p) m -> p i m", p=P)
    del f32r  # unused in the all-fp16 variant

    with tile.TileContext(nc) as tc:
        with (
            tc.tile_pool(name="wpool", bufs=1) as wpool,
            tc.tile_pool(name="xpool", bufs=5) as xpool,
            tc.tile_pool(name="opool", bufs=6) as opool,
            tc.tile_pool(name="psum", bufs=8, space="PSUM") as psum_pool,
        ):
            # W as 6 chunks of [128 (cn), 768 (om)], all in one SBUF tile.
            # Tiles are float32r (bit-identical to f32; matmul runs at full
            # rate with FP22 multiply) — BIR verifier requires the producer
            # of an fp32r matmul operand to be typed fp32r.
            # Loads go on the Sync HWDGE ring; stores on the Scalar HWDGE
            # ring so stores never head-of-line-block loads. W and the first
            # batch's x are loaded chunk-wise so the first matmul only waits
            # for chunk 0 of each (~0.6 MB) instead of the full 3.75 MB.
            # HAM warm-up: the PE boots throttled at K=4/8 (1.2 GHz) and only
            # unthrottles after ~3.4us of sustained busy. The data-starved
            # fill can't provide that, so without this the first ~8us of
            # real matmuls run at half clock. Dummy matmuls on a zeroed tile
            # during the otherwise-idle preamble window flip the clock gate
            # before the first real operand arrives.
            warm = wpool.tile([P, T], f16, tag="warm")
            nc.gpsimd.memset(warm[:], 0.0)
            for _ in range(9):
                wps = psum_pool.tile([P, T], f32, tag="ps")
                nc.tensor.matmul(wps[:], warm[:, :P], warm[:], start=True, stop=True)

            wt = wpool.tile([P, NBLK, OM], f16)
            xt0 = xpool.tile([P, NBLK, T], f16)
            xr0 = xs[0].rearrange("(i p) t -> p i t", p=P)
            for i in range(0, NBLK, 2):
                nc.sync.dma_start(wt[:, i:i + 2, :], wr[:, i:i + 2, :])
                nc.sync.dma_start(xt0[:, i:i + 2, :], xr0[:, i:i + 2, :])

            xts = [xt0]
            loads = [None]
            for b in range(1, BP):
                xt = xpool.tile([P, NBLK, T], f16, tag="xt0")
                xr = xs[b].rearrange("(i p) t -> p i t", p=P)
                loads.append(nc.sync.dma_start(xt[:], xr))
                xts.append(xt)

            for b in range(BP):
                xt = xts[b]
                ot = opool.tile([P, NBLK, T], f32)
                orr = out[b].rearrange("(j p) t -> p j t", p=P)
                # Hold batch b's stores until the load of batch b+2 completes:
                # loads sprint at full HBM rate early instead of round-robin
                # sharing with stores, so the last x arrives well before the
                # PE needs it; the store backlog drains in the tail where HBM
                # would otherwise idle. opool=6 keeps copies/PSUM unblocked.
                dep = loads[b + 2] if b + 2 < BP else None
                for j in range(NBLK):
                    ps = psum_pool.tile([P, T], f32)
                    for i in range(NBLK):
                        nc.tensor.matmul(
                            ps[:],
                            wt[:, i, j * P:(j + 1) * P],
                            xt[:, i, :],
                            start=(i == 0),
                            stop=(i == NBLK - 1),
                        )
                    nc.vector.tensor_copy(ot[:, j, :], ps[:])
                    st = nc.scalar.dma_start(orr[:, j, :], ot[:, j, :])
                    if dep is not None:
                        _add_dep_helper(
                            st.ins, dep.ins, sync=True,
                            reason="hold stores behind prefetch loads",
                        )

    nc.compile()
    return nc


def _combined_operator(adj: np.ndarray, Theta: np.ndarray) -> np.ndarray:
    """W[(c,n),(o,m)] = sum_k Theta[k,c,o] * T[k,n,m], fp32, shape (768,768)."""
    adj = np.asarray(adj).astype(np.float32)
    Theta = np.asarray(Theta)
    d = adj.sum(axis=1)
    d_inv_sqrt = np.where(d > 0, 1.0 / np.sqrt(d), 0.0).astype(np.float32)
    L = (adj * d_inv_sqrt[None, :]).T * d_inv_sqrt[None, :]
    Ts = [np.eye(NV, dtype=np.float32), L.astype(np.float32)]
    for _ in range(2, K):
        Ts.append((2.0 * L @ Ts[-1] - Ts[-2]).astype(np.float32))
    Tcheb = np.stack(Ts[:K])  # (K, n, m)
    W = np.einsum("kco,knm->cnom", Theta.astype(np.float32), Tcheb)
    return np.ascontiguousarray(W.reshape(CN, OM), dtype=np.float16)


def kernel(x: np.ndarray, adj: np.ndarray, Theta: np.ndarray) -> np.ndarray:
    global _compiled_nc, last_result
    if _compiled_nc is None:
        _compiled_nc = _build_nc()
    nc = _compiled_nc

    W = _combined_operator(adj, Theta)
    # x: (64, 32, 24, 512) -> per-core shard [8, 768, 512], fp16 (the device
    # matmul consumes fp16 regardless; casting host-side halves HBM reads)
    xf = np.asarray(x).astype(np.float16).reshape(B, CN, T)
    in_maps = [
        {"xs": np.ascontiguousarray(xf[c * BP:(c + 1) * BP]), "w": W}
        for c in range(N_CORES)
    ]
    res = run_bass_kernel_spmd(nc, in_maps, core_ids=list(range(N_CORES)))
    last_result = res
    out = np.concatenate([r["out"] for r in res.results], axis=0)
    return np.ascontiguousarray(out.reshape(B, O, NV, T))
